# revision 1
# baseline (speedup 1.0000x reference)
"""GAT (2-layer dense graph attention) on 8 Trainium2 NeuronCores.

Sharding: nodes (rows) split 8 ways, 750 rows/core. Each core computes the
masked-softmax attention over all j for its rows and the aggregation, in a
column-on-partition ("transposed") layout so the attention probabilities feed
TensorE directly as the moving operand. One small AllGather (Wh2^T,
192KB/core) shares the second-layer projections between the two GAT layers.

Key tricks:
  - softmax max-subtraction is unnecessary here (logits are bounded dot
    products); a constant shift exp(e-SHIFT) centers the fp16 range instead.
  - exp(leaky_relu(s)) == max(exp(s), exp(0.2*s)): two Exp activations from
    the same ACT table set (no table switching) + one DVE max.
  - row-sums of the attention matrix via a ones-vector matmul fused into the
    PE stream; normalization deferred to after aggregation.
  - fp16 for Wh / probabilities / aggregation matmuls (1 cyc/row on PE),
    fp32 for the f1/f2 logit path. Validated vs numpy: ~3e-4 rel err.
"""

import sys
import numpy as np

sys.path.insert(0, "/opt/trn_rl_repo")

N = 6000
F_IN = 300
HID = 128
H = 8
NC = 8          # cores
R = 750         # rows per core
JT = 47         # j tiles of 128 (6016 padded)
NP = JT * 128   # 6016
KP = 384        # padded F_IN (3 chunks of 128)
NH = H * HID    # 1024
ALPHA = 0.2
SHIFT1 = 5.0    # exp shift layer 1
SHIFT2 = 8.0    # exp shift layer 2

_CACHE = {}


def _mm_acc(nc, psum, lhsT, rhs, start, stop, width=512):
    """Accumulating matmul with the moving operand split into <=512-column
    chunks so each matmul instruction writes a single PSUM bank."""
    n = rhs.shape[-1]
    for lo in range(0, n, width):
        hi = min(lo + width, n)
        nc.tensor.matmul(psum[:, lo:hi], lhsT, rhs[:, lo:hi],
                         start=start, stop=stop)


def _bcast_row(bass, row_ap, parts=128):
    """AP that reads a [1, n] DRAM row replicated across `parts` partitions."""
    return bass.AP(tensor=row_ap.tensor, offset=row_ap.offset,
                   ap=[[0, parts]] + [list(d) for d in row_ap.ap[1:]])


def _build(reps=1):
    import concourse.bass as bass
    import concourse.tile as tile
    import concourse.tile_utils as tile_utils
    from concourse import bacc, mybir
    from concourse.masks import make_identity

    tile_utils.max_sbuf_usage = 206 * 1024

    f32, f16 = mybir.dt.float32, mybir.dt.float16
    AF = mybir.ActivationFunctionType
    ALU = mybir.AluOpType

    nc = bacc.Bacc("TRN2", target_bir_lowering=False, debug=False,
                   enable_asserts=False, num_devices=NC)

    xT32 = nc.dram_tensor("xT32", [KP, NP], f32, kind="ExternalInput")
    xT16 = nc.dram_tensor("xT16", [KP, NP], f16, kind="ExternalInput")
    xTloc = nc.dram_tensor("xTloc", [KP, R], f32, kind="ExternalInput")
    W16 = nc.dram_tensor("W16", [KP, NH], f16, kind="ExternalInput")
    WT32 = nc.dram_tensor("WT32", [HID, H, KP], f32, kind="ExternalInput")
    A12 = nc.dram_tensor("a12", [HID, H, 2], f32, kind="ExternalInput")
    AO1 = nc.dram_tensor("aout1", [HID, 1], f32, kind="ExternalInput")
    AO2 = nc.dram_tensor("aout2", [HID, 1], f16, kind="ExternalInput")
    WO16 = nc.dram_tensor("Wout16", [NH, HID], f16, kind="ExternalInput")
    ADJT = nc.dram_tensor("adjT", [NP, R], f16, kind="ExternalInput")
    OUT = nc.dram_tensor("out", [R, HID], f32, kind="ExternalOutput")

    with tile.TileContext(nc) as tc:
        for rep in range(reps):
            _body(nc, tc, bass, tile, mybir, f32, f16, AF, ALU, make_identity,
                  xT32, xT16, xTloc, W16, WT32, A12, AO1, AO2, WO16, ADJT, OUT,
                  pfx=f"r{rep}_" if reps > 1 else "")
    nc.compile()
    return nc


def _body(nc, tc, bass, tile, mybir, f32, f16, AF, ALU, make_identity,
          xT32, xT16, xTloc, W16, WT32, A12, AO1, AO2, WO16, ADJT, OUT,
          pfx=""):
    with tc.tile_pool(name=pfx + "persist", bufs=1) as persist, \
         tc.tile_pool(name=pfx + "dram", bufs=1, space="DRAM") as dram:

        ident32 = persist.tile([128, 128], f32)
        make_identity(nc, ident32)
        ident16 = persist.tile([128, 128], f16)
        nc.vector.tensor_copy(out=ident16, in_=ident32)
        ones16 = persist.tile([128, 1], f16)
        nc.vector.memset(ones16, 1.0)
        zero_b = persist.tile([128, 1], f32)
        nc.vector.memset(zero_b, 0.0)

        # per-j-tile bias columns for the two Exp passes of layer 1:
        #   fE1[:, jt, h] = f2_h[j] - SHIFT1      (for exp(s - SHIFT1))
        #   fE2[:, jt, h] = 0.2*f2_h[j] - SHIFT1  (for exp(0.2s - SHIFT1))
        fE1 = persist.tile([128, JT, H], f32)
        fE2 = persist.tile([128, JT, H], f32)
        f1loc = persist.tile([H, R], f32)

        adjT_sb = persist.tile([128, JT, R], f16)  # adj^T fp16, phases B..D

        aout1_sb = persist.tile([128, 1], f32)
        nc.sync.dma_start(out=aout1_sb, in_=AO1[:])
        aout2_sb = persist.tile([128, 1], f16)
        nc.sync.dma_start(out=aout2_sb, in_=AO2[:])

        f1d = dram.tile([H, R], f32)
        hcd = dram.tile([H, 128, R], f16)
        ccin = dram.tile([128, R], f16)
        ccout = dram.tile([NC, 128, R], f16)
        g1d = dram.tile([1, R], f32)
        rd = dram.tile([2, R], f32)

        # adj^T load (persist slot: Tile may schedule these early/overlapped)
        for jt in range(JT):
            nc.sync.dma_start(out=adjT_sb[:, jt, :],
                              in_=ADJT[jt * 128:(jt + 1) * 128, :])

        # ============ Phase A1: f-path (fp32) ============
        with tc.tile_pool(name=pfx + "a1", bufs=1) as a1, \
             tc.tile_pool(name=pfx + "a1ps", bufs=2, space="PSUM") as a1ps:
            xt32 = a1.tile([128, 3, NP], f32)
            for c3 in range(3):
                nc.sync.dma_start(out=xt32[:, c3, :],
                                  in_=xT32[c3 * 128:(c3 + 1) * 128, :])
            xtl = a1.tile([128, 3, R], f32)
            for c3 in range(3):
                nc.sync.dma_start(out=xtl[:, c3, :],
                                  in_=xTloc[c3 * 128:(c3 + 1) * 128, :])
            wt32 = a1.tile([128, H, KP], f32)
            nc.sync.dma_start(out=wt32, in_=WT32[:])
            a12_sb = a1.tile([128, H, 2], f32)
            nc.sync.dma_start(out=a12_sb, in_=A12[:])

            # Wa[k, 16]: cols 0..7 = per-head a1-projected W, 8..15 = a2
            wa = a1.tile([128, 3, 16], f32)
            for c3 in range(3):
                for h in range(H):
                    pwa = a1ps.tile([128, 2], f32, tag="pwa", bufs=1)
                    nc.tensor.matmul(pwa, wt32[:, h, c3 * 128:(c3 + 1) * 128],
                                     a12_sb[:, h, :], start=True, stop=True)
                    nc.any.tensor_copy(out=wa[:, c3, h:h + 1], in_=pwa[:, 0:1])
                    nc.any.tensor_copy(out=wa[:, c3, 8 + h:9 + h], in_=pwa[:, 1:2])

            # f^T [16, NP] = Wa^T @ x^T : rows 0..7 f1 per head, 8..15 f2
            fT = a1.tile([16, NP], f32)
            for ncol in range(0, NP, 512):
                w = min(512, NP - ncol)
                pf = a1ps.tile([16, 512], f32, tag="pf")
                for c3 in range(3):
                    nc.tensor.matmul(pf[:, :w], wa[:, c3, :],
                                     xt32[:, c3, ncol:ncol + w],
                                     start=(c3 == 0), stop=(c3 == 2))
                nc.any.tensor_copy(out=fT[:, ncol:ncol + w], in_=pf[:, :w])

            # f1 for this core's rows
            p1 = a1ps.tile([H, R], f32, tag="p1", bufs=1)
            for c3 in range(3):
                _mm_acc(nc, p1, wa[:, c3, 0:8], xtl[:, c3, :],
                        start=(c3 == 0), stop=(c3 == 2))
            nc.any.tensor_copy(out=f1loc, in_=p1)
            nc.sync.dma_start(out=f1d, in_=f1loc)

            # transpose f2 rows per j-tile and build the two bias tables
            for jt in range(JT):
                pt = a1ps.tile([128, 16], f32, tag="pt")
                nc.tensor.transpose(pt, fT[:, jt * 128:(jt + 1) * 128],
                                    ident32[:16, :16])
                nc.vector.tensor_scalar_add(fE1[:, jt, :], pt[:, 8:16], -SHIFT1)
                nc.vector.tensor_scalar(fE2[:, jt, :], pt[:, 8:16],
                                        ALPHA, -SHIFT1, ALU.mult, ALU.add)

        # ============ Phases A2+B under the whT scope ============
        with tc.tile_pool(name=pfx + "whp", bufs=1) as whp:
            whT = whp.tile([128, JT, NH], f16)   # Wh, j on partitions

            # --- A2: Wh (fp16), x^T streamed per j-tile ---
            with tc.tile_pool(name=pfx + "a2", bufs=3) as a2, \
                 tc.tile_pool(name=pfx + "a2w", bufs=1) as a2w, \
                 tc.tile_pool(name=pfx + "a2ps", bufs=3, space="PSUM") as a2ps:
                w16_sb = a2w.tile([128, 3, NH], f16)
                for c3 in range(3):
                    nc.sync.dma_start(out=w16_sb[:, c3, :],
                                      in_=W16[c3 * 128:(c3 + 1) * 128, :])
                for jt in range(JT):
                    xt16 = a2.tile([128, 3, 128], f16, tag="xt16")
                    for c3 in range(3):
                        nc.sync.dma_start(
                            out=xt16[:, c3, :],
                            in_=xT16[c3 * 128:(c3 + 1) * 128,
                                     jt * 128:(jt + 1) * 128])
                    for half in range(2):
                        pw = a2ps.tile([128, 512], f32, tag="pw")
                        for c3 in range(3):
                            nc.tensor.matmul(pw, xt16[:, c3, :],
                                             w16_sb[:, c3, half * 512:(half + 1) * 512],
                                             start=(c3 == 0), stop=(c3 == 2))
                        nc.any.tensor_copy(
                            out=whT[:, jt, half * 512:(half + 1) * 512], in_=pw)

            # --- B: layer-1 attention, heads outer ---
            with tc.tile_pool(name=pfx + "bf1", bufs=2) as bf1, \
                 tc.tile_pool(name=pfx + "brr", bufs=1) as brr, \
                 tc.tile_pool(name=pfx + "bt", bufs=2) as bt, \
                 tc.tile_pool(name=pfx + "bp", bufs=3) as bp, \
                 tc.tile_pool(name=pfx + "belu", bufs=1) as belu, \
                 tc.tile_pool(name=pfx + "bps", bufs=2, space="PSUM") as bps, \
                 tc.tile_pool(name=pfx + "brps", bufs=2, space="PSUM") as brps:
                for h in range(H):
                    f1rep = bf1.tile([128, R], f32, tag="f1rep")
                    nc.sync.dma_start(out=f1rep,
                                      in_=_bcast_row(bass, f1d[h:h + 1, :]))

                    psA = bps.tile([128, R], f32, tag="psA")
                    psR = brps.tile([1, R], f32, tag="psR")
                    for jt in range(JT):
                        t1 = bt.tile([128, R], f16, tag="t1")
                        nc.scalar.activation(out=t1, in_=f1rep, func=AF.Exp,
                                             bias=fE1[:, jt, h:h + 1], scale=1.0)
                        t2 = bt.tile([128, R], f16, tag="t2")
                        nc.scalar.activation(out=t2, in_=f1rep, func=AF.Exp,
                                             bias=fE2[:, jt, h:h + 1], scale=ALPHA)
                        u_t = bt.tile([128, R], f16, tag="u")
                        nc.vector.tensor_tensor(out=u_t, in0=t1, in1=t2,
                                                op=ALU.max)
                        p_t = bp.tile([128, R], f16, tag="p")
                        nc.vector.tensor_tensor(out=p_t, in0=u_t,
                                                in1=adjT_sb[:, jt, :], op=ALU.mult)
                        _mm_acc(nc, psA, whT[:, jt, h * 128:(h + 1) * 128], p_t,
                                start=(jt == 0), stop=(jt == JT - 1))
                        _mm_acc(nc, psR, ones16, p_t,
                                start=(jt == 0), stop=(jt == JT - 1))

                    # normalize + elu -> hcat^T chunk (fp16) -> DRAM
                    rt = belu.tile([1, R], f32, tag="rt")
                    nc.vector.reciprocal(out=rt, in_=psR)
                    nc.sync.dma_start(out=rd[0:1, :], in_=rt)
                    rrep = brr.tile([128, R], f32, tag="rrep")
                    nc.sync.dma_start(out=rrep, in_=_bcast_row(bass, rd[0:1, :]))
                    v_t = belu.tile([128, R], f32, tag="v")
                    nc.vector.tensor_tensor(out=v_t, in0=psA, in1=rrep,
                                            op=ALU.mult)
                    neg_t = belu.tile([128, R], f16, tag="neg")
                    nc.vector.tensor_scalar_min(neg_t, v_t, 0.0)
                    en_t = belu.tile([128, R], f16, tag="en")
                    nc.scalar.activation(out=en_t, in_=neg_t, func=AF.Exp,
                                         bias=zero_b, scale=1.0)
                    hc_t = belu.tile([128, R], f32, tag="hc")
                    nc.vector.scalar_tensor_tensor(out=hc_t, in0=v_t, scalar=0.0,
                                                   in1=en_t, op0=ALU.max,
                                                   op1=ALU.add)
                    hc16 = belu.tile([128, R], f16, tag="hc16")
                    nc.vector.tensor_scalar_add(hc16, hc_t, -1.0)
                    nc.sync.dma_start(out=hcd[h], in_=hc16)

        # ============ Phases C+D under the "late" scope ============
        with tc.tile_pool(name=pfx + "late", bufs=1) as late:
            wh2Tall = late.tile([128, NP], f16)
            wh2j = late.tile([128, JT, 128], f16)
            g2E1 = late.tile([128, JT], f32)
            g2E2 = late.tile([128, JT], f32)
            g1rep = late.tile([128, R], f32)

            # --- C: Wh2, gather, g1/g2 ---
            with tc.tile_pool(name=pfx + "c1", bufs=2) as c1, \
                 tc.tile_pool(name=pfx + "cw", bufs=1) as cw, \
                 tc.tile_pool(name=pfx + "cps", bufs=2, space="PSUM") as cps:
                wo_sb = cw.tile([128, H, HID], f16, tag="wo")
                for k8 in range(H):
                    nc.sync.dma_start(out=wo_sb[:, k8, :],
                                      in_=WO16[k8 * 128:(k8 + 1) * 128, :])
                psW2 = cps.tile([128, R], f32, tag="psW2", bufs=1)
                for k8 in range(H):
                    hc_sb = c1.tile([128, R], f16, tag="hcs")
                    nc.sync.dma_start(out=hc_sb, in_=hcd[k8])
                    _mm_acc(nc, psW2, wo_sb[:, k8, :], hc_sb,
                            start=(k8 == 0), stop=(k8 == H - 1))
                wh2T32 = cw.tile([128, R], f32, tag="w32")
                nc.any.tensor_copy(out=wh2T32, in_=psW2)
                wh2T16 = cw.tile([128, R], f16, tag="w16")
                nc.vector.tensor_copy(out=wh2T16, in_=psW2)

                # g1 (own rows) = a_out1^T @ Wh2^T_local
                psG1 = cps.tile([1, R], f32, tag="psG1", bufs=1)
                _mm_acc(nc, psG1, aout1_sb, wh2T32, start=True, stop=True)
                g1r = cw.tile([1, R], f32, tag="g1r")
                nc.any.tensor_copy(out=g1r, in_=psG1)
                nc.sync.dma_start(out=g1d, in_=g1r)
                nc.sync.dma_start(out=g1rep, in_=_bcast_row(bass, g1d[0:1, :]))

                # AllGather Wh2^T across the 8 cores
                nc.sync.dma_start(out=ccin, in_=wh2T16)
                nc.gpsimd.collective_compute(
                    "AllGather", mybir.AluOpType.bypass,
                    replica_groups=[list(range(NC))],
                    ins=[ccin.opt()], outs=[ccout.opt()])
                nc.vector.memset(wh2Tall[:, NC * R:], 0.0)
                for b in range(NC):
                    nc.sync.dma_start(out=wh2Tall[:, b * R:(b + 1) * R],
                                      in_=ccout[b])

                # per-j-tile Wh2 (back in row layout) + g2 bias tables
                for jt in range(JT):
                    ptj = cps.tile([128, 128], f16, tag="ptj")
                    nc.tensor.transpose(ptj, wh2Tall[:, jt * 128:(jt + 1) * 128],
                                        ident16)
                    nc.any.tensor_copy(out=wh2j[:, jt, :], in_=ptj)
                    pg2 = cps.tile([128, 1], f32, tag="pg2", bufs=1)
                    nc.tensor.matmul(pg2, wh2Tall[:, jt * 128:(jt + 1) * 128],
                                     aout2_sb, start=True, stop=True)
                    nc.vector.tensor_scalar_add(g2E1[:, jt:jt + 1], pg2, -SHIFT2)
                    nc.vector.tensor_scalar(g2E2[:, jt:jt + 1], pg2,
                                            ALPHA, -SHIFT2, ALU.mult, ALU.add)

            # --- D: layer-2 attention ---
            with tc.tile_pool(name=pfx + "dt", bufs=2) as dt_, \
                 tc.tile_pool(name=pfx + "dp", bufs=3) as dp, \
                 tc.tile_pool(name=pfx + "dfin", bufs=1) as dfin, \
                 tc.tile_pool(name=pfx + "dout", bufs=2) as dout, \
                 tc.tile_pool(name=pfx + "dps", bufs=1, space="PSUM") as dps, \
                 tc.tile_pool(name=pfx + "dops", bufs=2, space="PSUM") as dops:
                psA2 = dps.tile([128, R], f32, tag="psA2")
                psR2 = dps.tile([1, R], f32, tag="psR2")
                for jt in range(JT):
                    t1 = dt_.tile([128, R], f16, tag="t1")
                    nc.scalar.activation(out=t1, in_=g1rep, func=AF.Exp,
                                         bias=g2E1[:, jt:jt + 1], scale=1.0)
                    t2 = dt_.tile([128, R], f16, tag="t2")
                    nc.scalar.activation(out=t2, in_=g1rep, func=AF.Exp,
                                         bias=g2E2[:, jt:jt + 1], scale=ALPHA)
                    u2 = dt_.tile([128, R], f16, tag="u2")
                    nc.vector.tensor_tensor(out=u2, in0=t1, in1=t2, op=ALU.max)
                    p2 = dp.tile([128, R], f16, tag="p2")
                    nc.vector.tensor_tensor(out=p2, in0=u2,
                                            in1=adjT_sb[:, jt, :], op=ALU.mult)
                    _mm_acc(nc, psA2, wh2j[:, jt, :], p2,
                            start=(jt == 0), stop=(jt == JT - 1))
                    _mm_acc(nc, psR2, ones16, p2,
                            start=(jt == 0), stop=(jt == JT - 1))

                r2t = dfin.tile([1, R], f32, tag="r2t")
                nc.vector.reciprocal(out=r2t, in_=psR2)
                nc.sync.dma_start(out=rd[1:2, :], in_=r2t)
                r2rep = dfin.tile([128, R], f32, tag="r2rep")
                nc.sync.dma_start(out=r2rep, in_=_bcast_row(bass, rd[1:2, :]))
                o_t = dfin.tile([128, R], f32, tag="o")
                nc.vector.tensor_tensor(out=o_t, in0=psA2, in1=r2rep,
                                        op=ALU.mult)

                # transpose back to row layout and write out
                for it in range(6):
                    w = min(128, R - it * 128)
                    po = dops.tile([128, 128], f32, tag="po")
                    nc.tensor.transpose(po[:w, :],
                                        o_t[:, it * 128:it * 128 + w], ident32)
                    orow = dout.tile([128, 128], f32, tag="orow")
                    nc.any.tensor_copy(out=orow[:w, :], in_=po[:w, :])
                    nc.sync.dma_start(out=OUT[it * 128:it * 128 + w, :],
                                      in_=orow[:w, :])


def _host_prep(x, adj, W_heads, a_heads, W_out, a_out):
    """Per-core input maps. Layout/pad/cast only — no model math."""
    xT = np.zeros((KP, NP), np.float32)
    xT[:F_IN, :N] = x.T
    xT16 = xT.astype(np.float16)
    W16 = np.zeros((KP, NH), np.float16)
    W16[:F_IN] = W_heads.transpose(1, 0, 2).reshape(F_IN, NH).astype(np.float16)
    WT32 = np.zeros((HID, H, KP), np.float32)
    WT32[:, :, :F_IN] = W_heads.transpose(2, 0, 1)
    a12 = np.stack([a_heads[:, :HID, 0], a_heads[:, HID:, 0]], axis=2)  # [H,HID,2]
    a12 = np.ascontiguousarray(a12.transpose(1, 0, 2))                  # [HID,H,2]
    ao1 = np.ascontiguousarray(a_out[:HID]).astype(np.float32)
    ao2 = np.ascontiguousarray(a_out[HID:]).astype(np.float16)
    Wout16 = W_out.astype(np.float16)

    in_maps = []
    for c in range(NC):
        rows = slice(c * R, (c + 1) * R)
        adjT = np.zeros((NP, R), np.float16)
        adjT[:N, :] = adj[rows].T
        xTloc = np.zeros((KP, R), np.float32)
        xTloc[:F_IN] = x[rows].T
        in_maps.append({
            "xT32": xT, "xT16": xT16, "xTloc": xTloc, "W16": W16,
            "WT32": WT32, "a12": a12, "aout1": ao1, "aout2": ao2,
            "Wout16": Wout16, "adjT": np.ascontiguousarray(adjT),
        })
    return in_maps


def run(inputs, trace=False, **kw):
    from concourse.bass_utils import run_bass_kernel_spmd
    if "nc" not in _CACHE:
        _CACHE["nc"] = _build()
    nc = _CACHE["nc"]
    in_maps = _host_prep(**inputs)
    res = run_bass_kernel_spmd(nc, in_maps, core_ids=list(range(NC)),
                               trace=trace, **kw)
    out = np.concatenate([res.results[c]["out"] for c in range(NC)], axis=0)
    return out, res


def kernel(x, adj, W_heads, a_heads, W_out, a_out):
    out, _ = run(dict(x=np.asarray(x), adj=np.asarray(adj),
                      W_heads=np.asarray(W_heads), a_heads=np.asarray(a_heads),
                      W_out=np.asarray(W_out), a_out=np.asarray(a_out)))
    return out

